# revision 1
# baseline (speedup 1.0000x reference)
"""Trainium2 Bass kernel for the NodeEdge GNN message-passing module.

Computes  out[b,n,h] = sum_e (w*inci + b)[n,e] * relu(inputs @ W_xes + b_xes)[b,e,h]
with B=16, N=2048, E=8192, DIM=64, DH=32.

Strategy: shard the edge (contraction) dimension E across the 8 NeuronCores
(EC=1024 edges per core). Each core:
  - computes xe = relu(inputs[:, e_shard, :] @ W_xes) for its edge shard
    in [e, (b,h)] layout via small PE matmuls,
  - forms A^T chunks (w * inci, transposed so e is the partition axis;
    the transpose itself is done on the host as layout prep),
  - runs the big matmul  out_partial[(b,h), n] = xe^T @ A^T  with fp32r
    (FP22 reduced-precision, full PE rate) accumulating in f32 PSUM.
Partial outputs (one per core) are summed on the host.

inci is shipped as uint8 (2 MiB of HBM traffic per core instead of 8),
cast to f32 on the vector engine, and multiplied into the w chunks in
place. Matmul operands are declared float32r end to end (the BIR
verifier requires fp32r matmul inputs to be produced as fp32r).
"""

from contextlib import ExitStack

import numpy as np

import concourse.bass as bass
import concourse.mybir as mybir
import concourse.tile as tile
from concourse import bacc
from concourse.bass_utils import run_bass_kernel_spmd

B, N, E, DIM = 16, 2048, 8192, 64
DH = DIM // 2              # 32
NCORES = 8
EC = E // NCORES           # 1024 edges per core
KC = EC // 128             # 8 e-chunks of 128
BH = B * DH                # 512 (flattened (b, h) output dim)
NB = N // 512              # 4 column blocks of the big matmul
NJ = B // 2                # 8 input tiles, two batch rows packed per tile

F32 = mybir.dt.float32
F32R = mybir.dt.float32r
U8 = mybir.dt.uint8

_PROGRAMS: dict = {}


def _build_program(with_bxes: bool, with_b: bool):
    nc = bacc.Bacc(
        "TRN2", target_bir_lowering=False, debug=False, enable_asserts=False
    )

    inp_t = nc.dram_tensor("inp_t", [NJ, 128, EC], F32R, kind="ExternalInput").ap()
    wq = nc.dram_tensor("wq", [KC, 128, N], F32R, kind="ExternalInput").ap()
    iq = nc.dram_tensor("iq", [KC, 128, N], U8, kind="ExternalInput").ap()
    wx = nc.dram_tensor("wx", [128, 2 * DH], F32R, kind="ExternalInput").ap()
    bxr = (
        nc.dram_tensor("bxr", [128, BH], F32, kind="ExternalInput").ap()
        if with_bxes
        else None
    )
    bq = (
        nc.dram_tensor("bq", [KC, 128, N], F32, kind="ExternalInput").ap()
        if with_b
        else None
    )
    outp = nc.dram_tensor("outp", [BH, N], F32, kind="ExternalOutput").ap()

    with tile.TileContext(nc) as tc, ExitStack() as ctx:
        inp_pool = ctx.enter_context(tc.tile_pool(name="inp", bufs=NJ))
        wx_pool = ctx.enter_context(tc.tile_pool(name="wx", bufs=1))
        xe_pool = ctx.enter_context(tc.tile_pool(name="xe", bufs=KC))
        a_pool = ctx.enter_context(tc.tile_pool(name="a", bufs=KC))
        i_pool = ctx.enter_context(tc.tile_pool(name="i", bufs=3))
        out_pool = ctx.enter_context(tc.tile_pool(name="o", bufs=8))
        ps_pool = ctx.enter_context(tc.tile_pool(name="ps", bufs=8, space="PSUM"))

        # Block-diagonal xes weight: rows 0-63 map the even batch row to
        # output cols 0-31, rows 64-127 map the odd batch row to cols
        # 32-63, so one K=128 matmul computes xe for both packed batch
        # rows of an input tile at once.
        wx_tile = wx_pool.tile([128, 2 * DH], F32R)
        nc.sync.dma_start(wx_tile[:], wx[:])

        # incidence loads first: they are small and the A^T chain needs
        # them early (cast runs while PE is busy with xe)
        iu_tiles = []
        for k in range(KC):
            iu = i_pool.tile([128, N], U8, tag="iu", name=f"iu_{k}", bufs=KC)
            nc.gpsimd.dma_start(iu[:], iq[k])
            iu_tiles.append(iu)

        bx_tile = None
        if with_bxes:
            bx_tile = wx_pool.tile([128, BH], F32, tag="bx")
            nc.sync.dma_start(bx_tile[:], bxr[:])

        # ---- load inputs (two batch rows packed per 128-partition tile) ----
        inp_tiles = []
        for j in range(NJ):
            t = inp_pool.tile([128, EC], F32R)
            nc.sync.dma_start(t[:], inp_t[j])
            inp_tiles.append(t)

        # ---- xe = relu(inputs @ W_xes) in [e, (b,h)] layout ----
        xe_tiles = []
        for k in range(KC):
            ps = ps_pool.tile([128, BH], F32, tag="ps")
            for j in range(NJ):
                lhsT = inp_tiles[j][:, k * 128 : (k + 1) * 128]
                nc.tensor.matmul(
                    ps[:, j * 2 * DH : (j + 1) * 2 * DH],
                    lhsT,
                    wx_tile[:],
                    start=True,
                    stop=True,
                )
            xt = xe_pool.tile([128, BH], F32R)
            if with_bxes:
                nc.vector.tensor_tensor(
                    xt[:], ps[:], bx_tile[:], op=mybir.AluOpType.add
                )
                nc.scalar.activation(
                    xt[:], xt[:], mybir.ActivationFunctionType.Relu
                )
            else:
                nc.scalar.activation(
                    xt[:], ps[:], mybir.ActivationFunctionType.Relu
                )
            xe_tiles.append(xt)

        # ---- A^T chunks: w, then *= inci. The u8 -> f32 casts run on
        # GpSimd (otherwise idle) so the DVE FIFO carries only the
        # w-DMA-paced multiply chain, and they are emitted chunks ahead.
        # Each chunk is split into NB quarter-tiles (one per output
        # column block) so multiplies and dependent matmuls start on the
        # first quarter while the rest of the w DMA is still in flight.
        QN = N // NB  # 512, one tile per output column block
        a_tiles = []
        it_tiles = [None] * KC
        for k in range(KC):
            ah = [
                a_pool.tile([128, QN], F32R, tag="a", bufs=NB * KC,
                            name=f"a_{k}_{h}")
                for h in range(NB)
            ]
            for h in range(NB):
                nc.sync.dma_start(ah[h][:], wq[k][:, h * QN : (h + 1) * QN])
            a_tiles.append(ah)

        def emit_cast(k):
            it = i_pool.tile([128, N], F32, tag="it", name=f"it_{k}", bufs=3)
            nc.gpsimd.tensor_copy(it[:], iu_tiles[k][:])
            it_tiles[k] = it

        emit_cast(0)
        if KC > 1:
            emit_cast(1)
        if KC > 2:
            emit_cast(2)
        for k in range(KC):
            for h in range(NB):
                nc.vector.tensor_tensor(
                    a_tiles[k][h][:], a_tiles[k][h][:],
                    it_tiles[k][:, h * QN : (h + 1) * QN],
                    op=mybir.AluOpType.mult,
                )
            if k + 3 < KC:
                emit_cast(k + 3)
            if with_b:
                bt = i_pool.tile([128, N], F32, tag="bt", bufs=2)
                nc.sync.dma_start(bt[:], bq[k])
                for h in range(NB):
                    nc.vector.tensor_tensor(
                        a_tiles[k][h][:], a_tiles[k][h][:],
                        bt[:, h * QN : (h + 1) * QN],
                        op=mybir.AluOpType.add,
                    )

        # ---- big matmul: out[(b,h), n] += xe^T @ A^T, fp32r, f32 accum ----
        for pair in range(BH // 256):  # two (b,h) 128-chunks at a time
            pstiles = [
                [
                    ps_pool.tile(
                        [128, 512], F32, tag="ps", name=f"bps_{pair}_{h2}_{nb}"
                    )
                    for nb in range(NB)
                ]
                for h2 in range(2)
            ]
            if pair == 0:
                # arrival-paced: walk k outermost so each chunk is used
                # as soon as its A^T half lands
                for k in range(KC):
                    for half in range(2):
                        bh = 2 * pair + half
                        lhsT = xe_tiles[k][:, bh * 128 : (bh + 1) * 128]
                        for nb in range(NB):
                            nc.tensor.matmul(
                                pstiles[half][nb][:],
                                lhsT,
                                a_tiles[k][nb][:],
                                start=(k == 0),
                                stop=(k == KC - 1),
                            )
            else:
                # all data has arrived by now: walk half outermost so the
                # first 4 groups close early and their copies/stores
                # overlap the remaining matmuls
                for half in range(2):
                    bh = 2 * pair + half
                    for nb in range(NB):
                        for k in range(KC):
                            nc.tensor.matmul(
                                pstiles[half][nb][:],
                                xe_tiles[k][:, bh * 128 : (bh + 1) * 128],
                                a_tiles[k][nb][:],
                                start=(k == 0),
                                stop=(k == KC - 1),
                            )
                    for nb in range(NB):
                        ot = out_pool.tile(
                            [128, 512], F32, tag="o", name=f"ot1_{half}_{nb}"
                        )
                        if nb % 2 == 0:
                            nc.scalar.activation(
                                ot[:],
                                pstiles[half][nb][:],
                                mybir.ActivationFunctionType.Identity,
                            )
                        else:
                            nc.vector.tensor_copy(ot[:], pstiles[half][nb][:])
                        nc.scalar.dma_start(
                            outp[bh * 128 : (bh + 1) * 128,
                                 nb * 512 : (nb + 1) * 512],
                            ot[:],
                        )
                continue
            for half in range(2):
                bh = 2 * pair + half
                for nb in range(NB):
                    ot = out_pool.tile([128, 512], F32, tag="o")
                    if nb % 2 == 0:
                        nc.scalar.activation(
                            ot[:],
                            pstiles[half][nb][:],
                            mybir.ActivationFunctionType.Identity,
                        )
                    else:
                        nc.vector.tensor_copy(ot[:], pstiles[half][nb][:])
                    nc.sync.dma_start(
                        outp[bh * 128 : (bh + 1) * 128, nb * 512 : (nb + 1) * 512],
                        ot[:],
                    )

    nc.compile()
    return nc


def _get_program(with_bxes: bool, with_b: bool):
    key = (with_bxes, with_b)
    if key not in _PROGRAMS:
        _PROGRAMS[key] = _build_program(with_bxes, with_b)
    return _PROGRAMS[key]


def _prepare_in_maps(inputs, W_xes, b_xes, inci, w, b, with_bxes, with_b):
    inputs = np.ascontiguousarray(np.asarray(inputs, dtype=np.float32))
    W_xes = np.ascontiguousarray(np.asarray(W_xes, dtype=np.float32))
    b_xes = np.asarray(b_xes, dtype=np.float32)
    w = np.asarray(w, dtype=np.float32)
    b = np.asarray(b, dtype=np.float32)
    inci_u8 = np.asarray(inci).astype(np.uint8)

    wx_dup = np.zeros((128, 2 * DH), dtype=np.float32)
    wx_dup[0:DIM, 0:DH] = W_xes
    wx_dup[DIM : 2 * DIM, DH : 2 * DH] = W_xes
    bxr = np.ascontiguousarray(
        np.broadcast_to(np.tile(b_xes, B)[None, :], (128, BH))
    ) if with_bxes else None

    in_maps = []
    for c in range(NCORES):
        sl = slice(c * EC, (c + 1) * EC)
        t = np.ascontiguousarray(inputs[:, sl, :].transpose(0, 2, 1)).reshape(
            NJ, 128, EC
        )
        wq_ = np.ascontiguousarray(w[:, sl].T).reshape(KC, 128, N)
        iq_ = np.ascontiguousarray(inci_u8[:, sl].T).reshape(KC, 128, N)
        m = {"inp_t": t, "wq": wq_, "iq": iq_, "wx": wx_dup}
        if with_bxes:
            m["bxr"] = bxr
        if with_b:
            m["bq"] = np.ascontiguousarray(b[:, sl].T).reshape(KC, 128, N)
        in_maps.append(m)
    return in_maps


def _run(inputs, W_xes, b_xes, inci, w, b, **run_kwargs):
    with_bxes = bool(np.any(np.asarray(b_xes)))
    with_b = bool(np.any(np.asarray(b)))
    nc = _get_program(with_bxes, with_b)
    in_maps = _prepare_in_maps(inputs, W_xes, b_xes, inci, w, b, with_bxes, with_b)
    res = run_bass_kernel_spmd(
        nc, in_maps, core_ids=list(range(NCORES)), **run_kwargs
    )
    parts = np.stack([r["outp"] for r in res.results])  # [8, BH, N]
    out = parts.sum(axis=0)  # [BH, N]
    out = out.reshape(B, DH, N).transpose(0, 2, 1)  # [B, N, DH]
    return np.ascontiguousarray(out.astype(np.float32)), res


def kernel(inputs, W_xes, b_xes, inci, w, b):
    out, _ = _run(inputs, W_xes, b_xes, inci, w, b)
    return out



# revision 2
# speedup vs baseline: 1.7229x; 1.7229x over previous
"""Trainium2 Bass kernel for the NodeEdge GNN message-passing module.

Computes  out[b,n,h] = sum_e (w*inci + b)[n,e] * relu(inputs @ W_xes + b_xes)[b,e,h]
with B=16, N=2048, E=8192, DIM=64, DH=32.

Strategy: shard the edge (contraction) dimension E across the 8 NeuronCores
(EC=1024 edges per core); partial outputs are summed on the host.

The whole on-device datapath is bf16 (the correctness gate is rel_err <
2e-2; bf16 quantization lands ~2e-3):
  - inputs / w / W_xes ship as bf16 (halves HBM traffic vs f32),
  - inci ships as uint8 and is cast to bf16 *inside* the SWDGE DMA
    (gpsimd-initiated DMAs cast in the datapath at line rate, so no
    engine ever touches the raw u8),
  - xe = relu(inputs @ W_xes) computed via PE matmuls, relu on ScalarE
    casting PSUM f32 -> bf16,
  - A = w*inci via one 2x-mode DVE multiply per 128-row e-chunk,
  - big matmul out[(b,h), n] += xe^T @ A^T in bf16, f32 PSUM accum,
  - output partials stored as bf16.

PSUM budget (8 banks) forces a two-phase big matmul: the first two
(b,h) 128-chunks accumulate chunk-by-chunk as A^T chunks arrive
(8 banks), the last two replay the (by then SBUF-resident) chunks
densely.  A short burst of dummy matmuls at t=0 warms the PE HAM
clock-gate so the real stream runs at 2.4 GHz.
"""

from contextlib import ExitStack

import ml_dtypes
import numpy as np

import concourse.bass as bass
import concourse.mybir as mybir
import concourse.tile as tile
from concourse import bacc
from concourse.bass_utils import run_bass_kernel_spmd

B, N, E, DIM = 16, 2048, 8192, 64
DH = DIM // 2              # 32
NCORES = 8
EC = E // NCORES           # 1024 edges per core
KC = EC // 128             # 8 e-chunks of 128
BH = B * DH                # 512 (flattened (b, h) output dim)
NB = N // 512              # 4 column blocks of the big matmul
NJ = B // 2                # 8 input tiles, two batch rows packed per tile

F32 = mybir.dt.float32
BF16 = mybir.dt.bfloat16
U8 = mybir.dt.uint8
BF16NP = ml_dtypes.bfloat16

_PROGRAMS: dict = {}


def _build_program(with_bxes: bool, with_b: bool):
    nc = bacc.Bacc(
        "TRN2", target_bir_lowering=False, debug=False, enable_asserts=False
    )

    inp_t = nc.dram_tensor("inp_t", [NJ, 128, EC], BF16, kind="ExternalInput").ap()
    wq = nc.dram_tensor("wq", [KC, 128, N], BF16, kind="ExternalInput").ap()
    iq = nc.dram_tensor("iq", [KC, 128, N], U8, kind="ExternalInput").ap()
    wx = nc.dram_tensor("wx", [128, 2 * DH], BF16, kind="ExternalInput").ap()
    bxr = (
        nc.dram_tensor("bxr", [128, BH], F32, kind="ExternalInput").ap()
        if with_bxes
        else None
    )
    bq = (
        nc.dram_tensor("bq", [KC, 128, N], BF16, kind="ExternalInput").ap()
        if with_b
        else None
    )
    outp = nc.dram_tensor("outp", [BH, N], BF16, kind="ExternalOutput").ap()

    with tile.TileContext(nc) as tc, ExitStack() as ctx:
        inp_pool = ctx.enter_context(tc.tile_pool(name="inp", bufs=NJ))
        wx_pool = ctx.enter_context(tc.tile_pool(name="wx", bufs=1))
        xe_pool = ctx.enter_context(tc.tile_pool(name="xe", bufs=KC))
        a_pool = ctx.enter_context(tc.tile_pool(name="a", bufs=KC))
        i_pool = ctx.enter_context(tc.tile_pool(name="i", bufs=4))
        out_pool = ctx.enter_context(tc.tile_pool(name="o", bufs=4))
        ps_pool = ctx.enter_context(tc.tile_pool(name="ps", bufs=8, space="PSUM"))

        # Block-diagonal xes weight: rows 0-63 map the even batch row to
        # output cols 0-31, rows 64-127 map the odd batch row to cols
        # 32-63, so one K=128 matmul computes xe for both packed batch
        # rows of an input tile at once.
        wx_tile = wx_pool.tile([128, 2 * DH], BF16)
        nc.sync.dma_start(wx_tile[:], wx[:])

        bx_tile = None
        if with_bxes:
            bx_tile = wx_pool.tile([128, BH], F32, tag="bx")
            nc.sync.dma_start(bx_tile[:], bxr[:])

        # ---- HAM warmup: keep the PE busy from t=0 so the clock gate is
        # at 8/8 by the time the real matmuls start. Writes a scratch
        # region of the psum bank that xe chunk 0 will reuse much later.
        ps_warm = ps_pool.tile([128, BH], F32, tag="ps", name="ps_warm")
        for i in range(36):
            nc.tensor.matmul(
                ps_warm[0:64, 0:64],
                wx_tile[:, 0:64],
                wx_tile[:, 0:64],
                start=True,
                stop=True,
            )

        # ---- loads. Per-engine FIFO order is the scheduling knob: inp
        # tiles first on the sync queue (xe needs all of them), then the
        # A^T chunks in k order. inci goes on the gpsimd (SWDGE) queue,
        # casting u8 -> bf16 inside the DMA.
        inp_tiles = []
        for j in range(NJ):
            t = inp_pool.tile([128, EC], BF16)
            nc.sync.dma_start(t[:], inp_t[j])
            inp_tiles.append(t)

        it_tiles = []
        for k in range(KC):
            it = i_pool.tile([128, N], BF16, tag="it", name=f"it_{k}", bufs=4)
            nc.gpsimd.dma_start(it[:], iq[k])
            it_tiles.append(it)

        a_tiles = []
        for k in range(KC):
            a = a_pool.tile([128, N], BF16, tag="a", name=f"a_{k}", bufs=KC)
            nc.sync.dma_start(a[:], wq[k])
            a_tiles.append(a)

        # ---- xe = relu(inputs @ W_xes) in [e, (b,h)] layout.
        # One PSUM bank per e-chunk; walk j outermost so each input tile
        # is consumed as soon as its DMA lands.
        ps_xe = [
            ps_pool.tile([128, BH], F32, tag="ps", name=f"ps_xe_{k}")
            for k in range(KC)
        ]
        for j in range(NJ):
            for k in range(KC):
                nc.tensor.matmul(
                    ps_xe[k][:, j * 2 * DH : (j + 1) * 2 * DH],
                    inp_tiles[j][:, k * 128 : (k + 1) * 128],
                    wx_tile[:],
                    start=True,
                    stop=True,
                )
        xe_tiles = []
        for k in range(KC):
            xt = xe_pool.tile([128, BH], BF16)
            if with_bxes:
                nc.vector.tensor_tensor(
                    xt[:], ps_xe[k][:], bx_tile[:], op=mybir.AluOpType.add
                )
                nc.scalar.activation(
                    xt[:], xt[:], mybir.ActivationFunctionType.Relu
                )
            else:
                nc.scalar.activation(
                    xt[:], ps_xe[k][:], mybir.ActivationFunctionType.Relu
                )
            xe_tiles.append(xt)

        # ---- A^T chunks: a[k] *= inci[k] (one full-width 2x-mode DVE
        # multiply per chunk).
        for k in range(KC):
            nc.vector.tensor_tensor(
                a_tiles[k][:], a_tiles[k][:], it_tiles[k][:],
                op=mybir.AluOpType.mult,
            )
            if with_b:
                bt = i_pool.tile([128, N], BF16, tag="bt", bufs=2)
                nc.sync.dma_start(bt[:], bq[k])
                nc.vector.tensor_tensor(
                    a_tiles[k][:], a_tiles[k][:], bt[:],
                    op=mybir.AluOpType.add,
                )

        # ---- big matmul: out[(b,h), n] += xe^T @ A^T, bf16, f32 accum.
        # Phase A: (b,h)-chunks 0-1 accumulate k-outermost (8 banks),
        # paced by chunk arrival.
        psA = [
            [
                ps_pool.tile([128, 512], F32, tag="ps", name=f"psA_{h}_{nb}")
                for nb in range(NB)
            ]
            for h in range(2)
        ]
        for k in range(KC):
            for h in range(2):
                lhsT = xe_tiles[k][:, h * 128 : (h + 1) * 128]
                for nb in range(NB):
                    nc.tensor.matmul(
                        psA[h][nb][:],
                        lhsT,
                        a_tiles[k][:, nb * 512 : (nb + 1) * 512],
                        start=(k == 0),
                        stop=(k == KC - 1),
                    )
        for h in range(2):
            ot = out_pool.tile([128, N], BF16, tag="o", name=f"otA_{h}")
            for nb in range(NB):
                if nb % 2 == 0:
                    nc.scalar.activation(
                        ot[:, nb * 512 : (nb + 1) * 512],
                        psA[h][nb][:],
                        mybir.ActivationFunctionType.Identity,
                    )
                else:
                    nc.vector.tensor_copy(
                        ot[:, nb * 512 : (nb + 1) * 512], psA[h][nb][:]
                    )
            nc.scalar.dma_start(outp[h * 128 : (h + 1) * 128, :], ot[:])

        # Phase B: (b,h)-chunks 2-3, all chunks SBUF-resident by now, so
        # walk h outermost and close each group early for overlap of the
        # copies/stores with the remaining matmuls.
        for h in range(2, 4):
            psB = [
                ps_pool.tile([128, 512], F32, tag="ps", name=f"psB_{h}_{nb}")
                for nb in range(NB)
            ]
            for nb in range(NB):
                for k in range(KC):
                    nc.tensor.matmul(
                        psB[nb][:],
                        xe_tiles[k][:, h * 128 : (h + 1) * 128],
                        a_tiles[k][:, nb * 512 : (nb + 1) * 512],
                        start=(k == 0),
                        stop=(k == KC - 1),
                    )
            ot = out_pool.tile([128, N], BF16, tag="o", name=f"otB_{h}")
            for nb in range(NB):
                if nb % 2 == 0:
                    nc.scalar.activation(
                        ot[:, nb * 512 : (nb + 1) * 512],
                        psB[nb][:],
                        mybir.ActivationFunctionType.Identity,
                    )
                else:
                    nc.vector.tensor_copy(
                        ot[:, nb * 512 : (nb + 1) * 512], psB[nb][:]
                    )
            nc.sync.dma_start(outp[h * 128 : (h + 1) * 128, :], ot[:])

    nc.compile()
    return nc


def _get_program(with_bxes: bool, with_b: bool):
    key = (with_bxes, with_b)
    if key not in _PROGRAMS:
        _PROGRAMS[key] = _build_program(with_bxes, with_b)
    return _PROGRAMS[key]


def _prepare_in_maps(inputs, W_xes, b_xes, inci, w, b, with_bxes, with_b):
    inputs = np.asarray(inputs, dtype=np.float32)
    W_xes = np.asarray(W_xes, dtype=np.float32)
    b_xes = np.asarray(b_xes, dtype=np.float32)
    w = np.asarray(w, dtype=np.float32)
    b = np.asarray(b, dtype=np.float32)
    inci_u8 = np.asarray(inci).astype(np.uint8)

    wx_dup = np.zeros((128, 2 * DH), dtype=np.float32)
    wx_dup[0:DIM, 0:DH] = W_xes
    wx_dup[DIM : 2 * DIM, DH : 2 * DH] = W_xes
    wx_dup = wx_dup.astype(BF16NP)
    bxr = np.ascontiguousarray(
        np.broadcast_to(np.tile(b_xes, B)[None, :], (128, BH))
    ) if with_bxes else None

    in_maps = []
    for c in range(NCORES):
        sl = slice(c * EC, (c + 1) * EC)
        t = np.ascontiguousarray(
            inputs[:, sl, :].transpose(0, 2, 1)
        ).reshape(NJ, 128, EC).astype(BF16NP)
        wq_ = np.ascontiguousarray(w[:, sl].T).reshape(KC, 128, N).astype(BF16NP)
        iq_ = np.ascontiguousarray(inci_u8[:, sl].T).reshape(KC, 128, N)
        m = {"inp_t": t, "wq": wq_, "iq": iq_, "wx": wx_dup}
        if with_bxes:
            m["bxr"] = bxr
        if with_b:
            m["bq"] = np.ascontiguousarray(b[:, sl].T).reshape(
                KC, 128, N
            ).astype(BF16NP)
        in_maps.append(m)
    return in_maps


def _run(inputs, W_xes, b_xes, inci, w, b, **run_kwargs):
    with_bxes = bool(np.any(np.asarray(b_xes)))
    with_b = bool(np.any(np.asarray(b)))
    nc = _get_program(with_bxes, with_b)
    in_maps = _prepare_in_maps(inputs, W_xes, b_xes, inci, w, b, with_bxes, with_b)
    res = run_bass_kernel_spmd(
        nc, in_maps, core_ids=list(range(NCORES)), **run_kwargs
    )
    parts = np.stack(
        [r["outp"].astype(np.float32) for r in res.results]
    )  # [8, BH, N]
    out = parts.sum(axis=0)  # [BH, N]
    out = out.reshape(B, DH, N).transpose(0, 2, 1)  # [B, N, DH]
    return np.ascontiguousarray(out.astype(np.float32)), res


def kernel(inputs, W_xes, b_xes, inci, w, b):
    out, _ = _run(inputs, W_xes, b_xes, inci, w, b)
    return out
